# Initial kernel scaffold
#
"""Sparse-attention Trainium2 kernel (nn_Attention_81398220193933).

Strategy (tensor-parallel over heads, 2 heads per NeuronCore):
  - Host pre-lays-out per-core tensors:
      qT  [B, 128, S]  bf16 : rows 0:64 = headA Q^T / sqrt(dh), rows 64:128 = headB
      kT  [B, 128, S]  bf16 : same for K^T
      vE  [B, 128, 8, 130] bf16 : per k-tile t, partition p = key position t*128+p,
           cols [0:64]=V_A*emb, [64]=emb, [65:129]=V_B*emb, [129]=emb
           where emb[b,k] = exp(bias[k]) * (k < seq_len[b]) (all-valid if seq_len==0).
    Folding the additive key bias + mask multiplicatively into V makes the
    softmax mask/bias free on-device and lets fully-masked k-tiles be skipped.
  - Device, per batch b and key-tile t (Kb = ceil(seq_len/128) tiles):
      scores^T [k=128, q=1024] = K_tile^T.T @ Q^T  (two heads packed into the
          PE array as 64-row tile_position groups -> run concurrently)
      W^T = exp(scores^T) on ScalarE (PSUM -> SBUF, bf16)
      out[q,65] += W^T_chunk.T @ V_tile  accumulated over t in PSUM; column 64
          accumulates the softmax denominator (via the emb column of vE).
    Epilogue per batch: denominators -> DVE reciprocal -> per-partition
    tensor_scalar multiply -> out [q, d] f32 -> DMA to HBM.
  - Softmax max-subtraction is unnecessary: logits are O(+-6) and masked keys
    contribute exactly zero through emb; a fully-masked row degenerates to
    softmax over all keys exactly like the jax reference (the -1e12 shift
    cancels there).
"""

import numpy as np
import ml_dtypes

import concourse.bass as bass
import concourse.mybir as mybir
import concourse.tile as tile
from concourse import bacc
from concourse.bass_utils import run_bass_kernel_spmd

B = 8
S = 1024
UNITS = 1024
H = 16
DH = 64
N_CORES = 8
KT = S // 128  # max key tiles per batch

BF16 = mybir.dt.bfloat16
F32 = mybir.dt.float32


def _build_nc(kbs):
    """Build the SPMD Bass program. kbs: per-batch number of 128-key tiles."""
    nc = bacc.Bacc("TRN2", target_bir_lowering=False, debug=False,
                   num_devices=N_CORES)
    qT = nc.dram_tensor("qt", [B, 128, S], BF16, kind="ExternalInput").ap()
    kT = nc.dram_tensor("kt", [B, 128, S], BF16, kind="ExternalInput").ap()
    vE = nc.dram_tensor("vt", [B, 128, KT, 130], BF16, kind="ExternalInput").ap()
    o = nc.dram_tensor("o", [B, S, 128], F32, kind="ExternalOutput").ap()

    with tile.TileContext(nc) as tc:
        with (
            tc.tile_pool(name="qk", bufs=2) as qk_pool,
            tc.tile_pool(name="v", bufs=2) as v_pool,
            tc.tile_pool(name="w", bufs=72) as w_pool,
            tc.tile_pool(name="ot", bufs=2) as o_pool,
            tc.tile_pool(name="rc", bufs=8) as r_pool,
            tc.tile_pool(name="sc", bufs=2, space="PSUM") as sc_pool,
            tc.tile_pool(name="acc", bufs=3, space="PSUM") as acc_pool,
            tc.tile_pool(name="junk", bufs=1, space="PSUM") as junk_pool,
        ):
            def emit_av(p, g):
                """Emit one A-V accumulation group (j-outer/t-inner keeps one
                PSUM group open per bank at a time). Groups 0-7 = head A,
                8-15 = head B; the head's two accumulator banks are allocated
                at its first group and divided/stored right after its last,
                so only 2 PSUM banks are held at any time."""
                h, j = divmod(g, 8)
                if j == 0:
                    p["acc"] = [
                        acc_pool.tile([128, 4, 65], F32, tag="acc",
                                      name=f"acc{p['b']}_{h}_{i}")
                        for i in range(2)]
                grp = p["acc"][j // 4]
                for t in range(p["kb"]):
                    nc.tensor.matmul(
                        grp[:, j % 4, :],
                        lhsT=p["wts"][h][t][:, j * 128:(j + 1) * 128],
                        rhs=p["vt"][:, t, h * 65:h * 65 + 65],
                        start=(t == 0), stop=(t == p["kb"] - 1),
                    )
                if j == 7:
                    epilogue(p, h)

            def epilogue(p, h):
                """Divide head h by its denominators and store."""
                ot = o_pool.tile([128, 8, 64], F32, tag="ot", name="ot")
                for g in range(2):
                    grp = p["acc"][g]
                    rc = r_pool.tile([128, 4, 1], F32, tag="rc", name="rc")
                    nc.vector.reciprocal(rc[:], grp[:, :, 64:65])
                    rc_b = bass.AP(tensor=rc.tensor, offset=rc.offset,
                                   ap=[rc.ap[0], rc.ap[1], [0, 64]])
                    nc.vector.tensor_tensor(
                        ot[:, 4 * g:4 * (g + 1), :],
                        grp[:, :, 0:64], rc_b, mybir.AluOpType.mult)
                ov = o[p["b"]].rearrange("(t p) c -> p t c", p=128)
                nc.sync.dma_start(
                    out=ov[:, :, h * 64:(h + 1) * 64], in_=ot[:])

            # HAM warmup + keep-warm machinery. The PE clock-gate (HAM)
            # only delivers 2.4 GHz while the PE looks busy; this kernel is
            # ScalarE-paced, so the PE's natural ~60% duty cycle makes HAM
            # oscillate back to 1.2 GHz. A startup burst warms it, and a few
            # dependency-free "junk" matmuls per key-tile keep it warm.
            wexp = qk_pool.tile([1, 8], F32, tag="wexp", name="wexp", bufs=1)
            nc.vector.memset(wexp[:], 0.0)
            # Preload the exp table-set (~2.7us) while the first DMAs fly.
            nc.scalar.activation(wexp[:], wexp[:],
                                 mybir.ActivationFunctionType.Exp)
            wu = qk_pool.tile([128, 640], BF16, tag="wu", name="wu")
            nc.vector.memset(wu[:], 0.0)

            # Load every batch's inputs up front (fits easily in SBUF) so no
            # QK phase ever waits on DMA. Process batches largest-first so
            # the post-last-exp tail (A-V + epilogue of the final batch) is
            # as short as possible. First batch's Q/K go first in the DMA
            # queue; V tiles are only needed one batch later.
            # First batch small (warms HAM on real work at low cost), then
            # largest-first, smallest last (short tail after the final exp).
            srt = sorted(range(B), key=lambda i: -kbs[i])
            order = [srt[-2]] + srt[:-2] + [srt[-1]]
            qts, kts, vts = {}, {}, {}
            for b in order:
                qts[b] = qk_pool.tile([128, S], BF16, tag=f"qt{b}",
                                      name=f"qt{b}", bufs=1)
                nc.sync.dma_start(out=qts[b][:], in_=qT[b])
                kts[b] = qk_pool.tile([128, S], BF16, tag=f"kt{b}",
                                      name=f"kt{b}", bufs=1)
                nc.sync.dma_start(out=kts[b][:], in_=kT[b])
            for b in order:
                vts[b] = v_pool.tile([128, kbs[b], 130], BF16, tag=f"vt{b}",
                                     name=f"vt{b}", bufs=1)
                nc.sync.dma_start(out=vts[b][:], in_=vE[b, :, :kbs[b], :])

            jk = junk_pool.tile([128, 512], F32, tag="junk", name="jk")

            def keep_warm(n):
                for _ in range(n):
                    nc.tensor.matmul(jk[:], lhsT=wu[:, 0:128],
                                     rhs=wu[:, 128:640],
                                     start=True, stop=True,
                                     skip_group_check=True)

            # Global step stream: one step per (batch, key-tile). A-V groups
            # of finished batches queue up and drip out at a fixed per-step
            # rate, so batch boundaries never pile PE work in front of the
            # next QK pair.
            avq = []  # (batch record, group) FIFO
            total_steps = sum(kbs)
            step_no = 0
            for bi, b in enumerate(order):
                kb = kbs[b]
                qt, kt, vt = qts[b], kts[b], vts[b]
                wts = [[], []]
                last = bi == len(order) - 1
                # Normal batches: per key-tile emit both heads' QK+exp.
                # Last batch: all of head A's tiles first, then head B's, so
                # head A's A-V (which needs every A exp) overlaps head B's
                # exp phase instead of extending the kernel tail.
                if last:
                    step_list = [(t, h) for h in range(2) for t in range(kb)]
                else:
                    step_list = [(t, h) for t in range(kb) for h in range(2)]
                rec = {"b": b, "kb": kb, "wts": wts, "vt": vt}
                for t, h in step_list:
                    base = 64 * h
                    sc = sc_pool.tile([128, S], F32, tag="sc", name="sc")
                    for qc in range(2):
                        nc.tensor.matmul(
                            sc[:, qc * 512:(qc + 1) * 512],
                            lhsT=kt[base:base + 64, t * 128:(t + 1) * 128],
                            rhs=qt[base:base + 64, qc * 512:(qc + 1) * 512],
                            start=True, stop=True,
                        )
                    wt = w_pool.tile([128, S], BF16, tag="w",
                                     name=f"w{b}_{t}_{h}")
                    nc.scalar.activation(wt[:], sc[:],
                                         mybir.ActivationFunctionType.Exp)
                    wts[h].append(wt)
                    if last and h == 0 and t == kb - 1:
                        # head A complete: its A-V can interleave from here
                        avq.extend((rec, g) for g in range(8))
                    # spread queued A-V groups over the remaining steps so the
                    # tail stays PE-dense (prevents late HAM re-throttle)
                    step_no += 0 if h else 1
                    rem = max(1, total_steps - step_no)
                    rate = -(-len(avq) // min(rem, 8))  # drain over ~8 steps
                    if h == 1 or last:
                        for _ in range(min(rate, 6)):
                            if avq:
                                emit_av(*avq.pop(0))
                        if last:
                            keep_warm(2)
                        else:
                            keep_warm(3 if len(avq) >= 8 else 5)
                avq.extend((rec, g) for g in range(8 if last else 0, 16))

            while avq:
                emit_av(*avq.pop(0))
                keep_warm(1)
    nc.compile()
    return nc


_NC_CACHE = {}


def _get_nc(kbs):
    key = tuple(kbs)
    if key not in _NC_CACHE:
        _NC_CACHE[key] = _build_nc(key)
    return _NC_CACHE[key]


def kernel(memory, query, b, seq_len):
    memory = np.asarray(memory)
    query = np.asarray(query)
    bias = np.asarray(b, dtype=np.float32)
    seq_len = np.asarray(seq_len).reshape(-1).astype(np.int64)

    sl = seq_len.copy()
    kbs = [int(min(KT, max(1, -(-int(s) // 128)))) if s > 0 else KT for s in sl]

    # emb[b, k] = exp(bias[k]) * valid; fully-masked batch -> plain softmax
    pos = np.arange(S)[None, :]
    valid = (pos < sl[:, None]) | (sl[:, None] == 0)
    emb = np.exp(bias)[None, :] * valid.astype(np.float32)  # [B, S]

    qh = (query.astype(np.float32) * (DH ** -0.5)).reshape(B, S, H, DH)
    kh = memory[:, :, :UNITS].astype(np.float32).reshape(B, S, H, DH)
    vh = memory[:, :, UNITS:].astype(np.float32).reshape(B, S, H, DH)
    vh = vh * emb[:, :, None, None]  # [B, S, H, DH] value rows pre-masked

    bf = ml_dtypes.bfloat16
    # [B, S, H, DH] -> [B, H, DH, S] transposed layouts
    qTfull = np.ascontiguousarray(qh.transpose(0, 2, 3, 1)).astype(bf)
    kTfull = np.ascontiguousarray(kh.transpose(0, 2, 3, 1)).astype(bf)
    # [B, S, H, DH] -> [B, (t p), H, DH] -> [B, 128, KT, H, DH]
    vtiles = np.ascontiguousarray(
        vh.reshape(B, KT, 128, H, DH).transpose(0, 2, 1, 3, 4)).astype(bf)
    embt = np.ascontiguousarray(
        emb.reshape(B, KT, 128).transpose(0, 2, 1)).astype(bf)  # [B, 128, KT]

    in_maps = []
    for c in range(N_CORES):
        hA, hB = 2 * c, 2 * c + 1
        qT = np.concatenate([qTfull[:, hA], qTfull[:, hB]], axis=1)  # [B,128,S]
        kT = np.concatenate([kTfull[:, hA], kTfull[:, hB]], axis=1)
        vE = np.empty((B, 128, KT, 130), dtype=bf)
        vE[..., 0:64] = vtiles[:, :, :, hA, :]
        vE[..., 64] = embt
        vE[..., 65:129] = vtiles[:, :, :, hB, :]
        vE[..., 129] = embt
        in_maps.append({
            "qt": np.ascontiguousarray(qT),
            "kt": np.ascontiguousarray(kT),
            "vt": np.ascontiguousarray(vE),
        })

    nc = _get_nc(kbs)
    res = run_bass_kernel_spmd(nc, in_maps, core_ids=list(range(N_CORES)))

    out = np.empty((B, S, UNITS), dtype=np.float32)
    for c in range(N_CORES):
        out[:, :, 128 * c:128 * (c + 1)] = res.results[c]["o"]
    return out



# revision 1
# speedup vs baseline: 1.0663x; 1.0663x over previous
"""Sparse-attention Trainium2 kernel (nn_Attention_81398220193933).

Strategy (tensor-parallel over heads, 2 heads per NeuronCore):
  - Host pre-lays-out per-core tensors:
      qT  [B, 128, S]  bf16 : rows 0:64 = headA Q^T / sqrt(dh), rows 64:128 = headB
      kT  [B, 128, S]  bf16 : same for K^T
      vE  [B, 128, 8, 130] bf16 : per k-tile t, partition p = key position t*128+p,
           cols [0:64]=V_A*emb, [64]=emb, [65:129]=V_B*emb, [129]=emb
           where emb[b,k] = exp(bias[k]) * (k < seq_len[b]) (all-valid if seq_len==0).
    Folding the additive key bias + mask multiplicatively into V makes the
    softmax mask/bias free on-device and lets fully-masked k-tiles be skipped.
  - Device, per batch b and key-tile t (Kb = ceil(seq_len/128) tiles):
      scores^T [k=128, q=1024] = K_tile^T.T @ Q^T  (two heads packed into the
          PE array as 64-row tile_position groups -> run concurrently)
      W^T = exp(scores^T) on ScalarE (PSUM -> SBUF, bf16)
      out[q,65] += W^T_chunk.T @ V_tile  accumulated over t in PSUM; column 64
          accumulates the softmax denominator (via the emb column of vE).
    Epilogue per batch: denominators -> DVE reciprocal -> per-partition
    tensor_scalar multiply -> out [q, d] f32 -> DMA to HBM.
  - Softmax max-subtraction is unnecessary: logits are O(+-6) and masked keys
    contribute exactly zero through emb; a fully-masked row degenerates to
    softmax over all keys exactly like the jax reference (the -1e12 shift
    cancels there).
"""

import numpy as np
import ml_dtypes

import concourse.bass as bass
import concourse.mybir as mybir
import concourse.tile as tile
from concourse import bacc
from concourse.bass_utils import run_bass_kernel_spmd

B = 8
S = 1024
UNITS = 1024
H = 16
DH = 64
N_CORES = 8
KT = S // 128  # max key tiles per batch

BF16 = mybir.dt.bfloat16
F32 = mybir.dt.float32


def _build_nc(kbs):
    """Build the SPMD Bass program. kbs: per-batch number of 128-key tiles."""
    nc = bacc.Bacc("TRN2", target_bir_lowering=False, debug=False,
                   num_devices=N_CORES)
    qT = nc.dram_tensor("qt", [B, 128, S], BF16, kind="ExternalInput").ap()
    kT = nc.dram_tensor("kt", [B, 128, S], BF16, kind="ExternalInput").ap()
    vE = nc.dram_tensor("vt", [B, 128, KT, 130], BF16, kind="ExternalInput").ap()
    o = nc.dram_tensor("o", [B, S, 128], F32, kind="ExternalOutput").ap()

    with tile.TileContext(nc) as tc:
        with (
            tc.tile_pool(name="qk", bufs=2) as qk_pool,
            tc.tile_pool(name="v", bufs=2) as v_pool,
            tc.tile_pool(name="w", bufs=72) as w_pool,
            tc.tile_pool(name="ot", bufs=2) as o_pool,
            tc.tile_pool(name="rc", bufs=8) as r_pool,
            tc.tile_pool(name="sc", bufs=2, space="PSUM") as sc_pool,
            tc.tile_pool(name="acc", bufs=3, space="PSUM") as acc_pool,
            tc.tile_pool(name="junk", bufs=1, space="PSUM") as junk_pool,
        ):
            def emit_av(p, g):
                """Emit one A-V accumulation group (j-outer/t-inner keeps one
                PSUM group open per bank at a time). Groups 0-7 = head A,
                8-15 = head B; the head's two accumulator banks are allocated
                at its first group and divided/stored right after its last,
                so only 2 PSUM banks are held at any time."""
                h, j = divmod(g, 8)
                if j == 0:
                    p["acc"] = [
                        acc_pool.tile([128, 4, 65], F32, tag="acc",
                                      name=f"acc{p['b']}_{h}_{i}")
                        for i in range(2)]
                grp = p["acc"][j // 4]
                for t in range(p["kb"]):
                    nc.tensor.matmul(
                        grp[:, j % 4, :],
                        lhsT=p["wts"][h][t][:, j * 128:(j + 1) * 128],
                        rhs=p["vt"][:, t, h * 65:h * 65 + 65],
                        start=(t == 0), stop=(t == p["kb"] - 1),
                    )
                if j == 7:
                    epilogue(p, h)

            def epilogue(p, h):
                """Divide head h by its denominators and store."""
                ot = o_pool.tile([128, 8, 64], F32, tag="ot", name="ot")
                for g in range(2):
                    grp = p["acc"][g]
                    rc = r_pool.tile([128, 4, 1], F32, tag="rc", name="rc")
                    nc.vector.reciprocal(rc[:], grp[:, :, 64:65])
                    rc_b = bass.AP(tensor=rc.tensor, offset=rc.offset,
                                   ap=[rc.ap[0], rc.ap[1], [0, 64]])
                    nc.vector.tensor_tensor(
                        ot[:, 4 * g:4 * (g + 1), :],
                        grp[:, :, 0:64], rc_b, mybir.AluOpType.mult)
                ov = o[p["b"]].rearrange("(t p) c -> p t c", p=128)
                nc.sync.dma_start(
                    out=ov[:, :, h * 64:(h + 1) * 64], in_=ot[:])

            # HAM warmup + keep-warm machinery. The PE clock-gate (HAM)
            # only delivers 2.4 GHz while the PE looks busy; this kernel is
            # ScalarE-paced, so the PE's natural ~60% duty cycle makes HAM
            # oscillate back to 1.2 GHz. A startup burst warms it, and a few
            # dependency-free "junk" matmuls per key-tile keep it warm.
            wexp = qk_pool.tile([1, 8], F32, tag="wexp", name="wexp", bufs=1)
            nc.vector.memset(wexp[:], 0.0)
            # Preload the exp table-set (~2.7us) while the first DMAs fly.
            nc.scalar.activation(wexp[:], wexp[:],
                                 mybir.ActivationFunctionType.Exp)
            wu = qk_pool.tile([128, 640], BF16, tag="wu", name="wu")
            nc.vector.memset(wu[:], 0.0)

            # Load every batch's inputs up front (fits easily in SBUF) so no
            # QK phase ever waits on DMA. Process batches largest-first so
            # the post-last-exp tail (A-V + epilogue of the final batch) is
            # as short as possible. First batch's Q/K go first in the DMA
            # queue; V tiles are only needed one batch later.
            # First batch small (warms HAM on real work at low cost), then
            # largest-first, smallest last (short tail after the final exp).
            srt = sorted(range(B), key=lambda i: -kbs[i])
            order = [srt[-2]] + srt[:-2] + [srt[-1]]
            qts, kts, vts = {}, {}, {}
            for b in order:
                qts[b] = qk_pool.tile([128, S], BF16, tag=f"qt{b}",
                                      name=f"qt{b}", bufs=1)
                nc.sync.dma_start(out=qts[b][:], in_=qT[b])
                kts[b] = qk_pool.tile([128, S], BF16, tag=f"kt{b}",
                                      name=f"kt{b}", bufs=1)
                nc.sync.dma_start(out=kts[b][:], in_=kT[b])
            for b in order:
                vts[b] = v_pool.tile([128, kbs[b], 130], BF16, tag=f"vt{b}",
                                     name=f"vt{b}", bufs=1)
                nc.sync.dma_start(out=vts[b][:], in_=vE[b, :, :kbs[b], :])

            jk = junk_pool.tile([128, 512], F32, tag="junk", name="jk")

            def keep_warm(n):
                for _ in range(n):
                    nc.tensor.matmul(jk[:], lhsT=wu[:, 0:128],
                                     rhs=wu[:, 128:640],
                                     start=True, stop=True,
                                     skip_group_check=True)

            # Global step stream: one step per (batch, key-tile). A-V groups
            # of finished batches queue up and drip out at a fixed per-step
            # rate, so batch boundaries never pile PE work in front of the
            # next QK pair.
            avq = []  # (batch record, group) FIFO
            total_steps = sum(kbs)
            step_no = 0
            for bi, b in enumerate(order):
                kb = kbs[b]
                qt, kt, vt = qts[b], kts[b], vts[b]
                wts = [[], []]
                last = bi == len(order) - 1
                # Normal batches: per key-tile emit both heads' QK+exp.
                # Last batch: all of head A's tiles first, then head B's, so
                # head A's A-V (which needs every A exp) overlaps head B's
                # exp phase instead of extending the kernel tail.
                if last:
                    step_list = [(t, h) for h in range(2) for t in range(kb)]
                else:
                    step_list = [(t, h) for t in range(kb) for h in range(2)]
                rec = {"b": b, "kb": kb, "wts": wts, "vt": vt}
                for t, h in step_list:
                    base = 64 * h
                    sc = sc_pool.tile([128, S], F32, tag="sc", name="sc")
                    for qc in range(2):
                        nc.tensor.matmul(
                            sc[:, qc * 512:(qc + 1) * 512],
                            lhsT=kt[base:base + 64, t * 128:(t + 1) * 128],
                            rhs=qt[base:base + 64, qc * 512:(qc + 1) * 512],
                            start=True, stop=True,
                        )
                    wt = w_pool.tile([128, S], BF16, tag="w",
                                     name=f"w{b}_{t}_{h}")
                    nc.scalar.activation(wt[:], sc[:],
                                         mybir.ActivationFunctionType.Exp)
                    wts[h].append(wt)
                    if last and h == 0 and t == kb - 1:
                        # head A complete: its A-V can interleave from here
                        avq.extend((rec, g) for g in range(8))
                    # spread queued A-V groups over the remaining steps so the
                    # tail stays PE-dense (prevents late HAM re-throttle)
                    step_no += 0 if h else 1
                    rem = max(1, total_steps - step_no)
                    rate = -(-len(avq) // min(rem, 8))  # drain over ~8 steps
                    if h == 1 or last:
                        for _ in range(min(rate, 6)):
                            if avq:
                                emit_av(*avq.pop(0))
                        if last:
                            keep_warm(2)
                        else:
                            keep_warm(3 if len(avq) >= 8 else 5)
                avq.extend((rec, g) for g in range(8 if last else 0, 16))

            while avq:
                emit_av(*avq.pop(0))
                keep_warm(1)
    nc.compile()
    return nc


_NC_CACHE = {}


def _get_nc(kbs):
    key = tuple(kbs)
    if key not in _NC_CACHE:
        _NC_CACHE[key] = _build_nc(key)
    return _NC_CACHE[key]


def kernel(memory, query, b, seq_len):
    memory = np.asarray(memory)
    query = np.asarray(query)
    bias = np.asarray(b, dtype=np.float32)
    seq_len = np.asarray(seq_len).reshape(-1).astype(np.int64)

    sl = seq_len.copy()
    kbs = [int(min(KT, max(1, -(-int(s) // 128)))) if s > 0 else KT for s in sl]

    # emb[b, k] = exp(bias[k]) * valid; fully-masked batch -> plain softmax
    pos = np.arange(S)[None, :]
    valid = (pos < sl[:, None]) | (sl[:, None] == 0)
    emb = np.exp(bias)[None, :] * valid.astype(np.float32)  # [B, S]

    qh = (query.astype(np.float32) * (DH ** -0.5)).reshape(B, S, H, DH)
    kh = memory[:, :, :UNITS].astype(np.float32).reshape(B, S, H, DH)
    vh = memory[:, :, UNITS:].astype(np.float32).reshape(B, S, H, DH)
    vh = vh * emb[:, :, None, None]  # [B, S, H, DH] value rows pre-masked

    bf = ml_dtypes.bfloat16
    # [B, S, H, DH] -> [B, H, DH, S] transposed layouts
    qTfull = np.ascontiguousarray(qh.transpose(0, 2, 3, 1)).astype(bf)
    kTfull = np.ascontiguousarray(kh.transpose(0, 2, 3, 1)).astype(bf)
    # [B, S, H, DH] -> [B, (t p), H, DH] -> [B, 128, KT, H, DH]
    vtiles = np.ascontiguousarray(
        vh.reshape(B, KT, 128, H, DH).transpose(0, 2, 1, 3, 4)).astype(bf)
    embt = np.ascontiguousarray(
        emb.reshape(B, KT, 128).transpose(0, 2, 1)).astype(bf)  # [B, 128, KT]

    in_maps = []
    for c in range(N_CORES):
        hA, hB = 2 * c, 2 * c + 1
        qT = np.concatenate([qTfull[:, hA], qTfull[:, hB]], axis=1)  # [B,128,S]
        kT = np.concatenate([kTfull[:, hA], kTfull[:, hB]], axis=1)
        vE = np.empty((B, 128, KT, 130), dtype=bf)
        vE[..., 0:64] = vtiles[:, :, :, hA, :]
        vE[..., 64] = embt
        vE[..., 65:129] = vtiles[:, :, :, hB, :]
        vE[..., 129] = embt
        in_maps.append({
            "qt": np.ascontiguousarray(qT),
            "kt": np.ascontiguousarray(kT),
            "vt": np.ascontiguousarray(vE),
        })

    nc = _get_nc(kbs)
    res = run_bass_kernel_spmd(nc, in_maps, core_ids=list(range(N_CORES)))

    out = np.empty((B, S, UNITS), dtype=np.float32)
    for c in range(N_CORES):
        out[:, :, 128 * c:128 * (c + 1)] = res.results[c]["o"]
    return out

